# revision 6
# baseline (speedup 1.0000x reference)
"""GNN message-passing kernel for 8 trn2 NeuronCores (Bass/Tile).

Model (reference):
    msg  = relu(concat(x[src], x[dst], e_attr) @ W_msg + b_msg)   # [E, 30]
    x1   = segment_sum(msg, dst, N)                                # [N, 30]
    h    = relu(x1 @ W1 + b1)                                      # [N, 20]
    g    = segment_sum(h, batch, G)                                # [G, 20]
    out  = relu(g @ W2 + b2) @ W3 + b3                             # [G, 1]

Sharding: edges are bucketed by dst range (12544 nodes per core, 98
128-node blocks per core, each block statically padded to 2304 edge
slots).  The host gathers raw node features per edge endpoint and ships
transposed bf16 streams; the device computes messages with two matmuls
per 128-edge tile (stacked src|dst features, eattr|bias), applies relu,
and scatter-adds by dst via a one-hot matmul into per-block PSUM.  The
per-node MLP + graph pooling run per block; per-graph partial sums are
scattered into a [GPAD, 20] buffer, AllReduced across the 8 cores, and
the tiny graph-level head is computed redundantly on every core.
"""
import sys

if "/opt/trn_rl_repo" not in sys.path:
    sys.path.insert(0, "/opt/trn_rl_repo")

import numpy as np
import ml_dtypes

bf16 = ml_dtypes.bfloat16

# ---------------------------------------------------------------- config

class Cfg:
    # model dims
    N = 100000          # nodes
    E = 1600000         # edges
    D = 64              # feature dim
    G = 1000            # graphs
    DM = 30             # message dim
    # sharding
    NCORES = 8
    NPC = 12544         # nodes per core (98 * 128)
    NBLK = 98           # 128-node blocks per core
    BS = 2304           # edge slots per block (18 tiles)
    TPB = 18            # tiles per block
    CH = 6              # tiles per chunk
    NCH = 3             # chunks per block
    GSPAN = 192         # per-core graph window (incl. trash slot GSPAN-1)
    GPAD = 1280         # padded global graph rows (1000 real + trash)

    @property
    def ES(self):       # edge slots per core
        return self.NBLK * self.BS

    @property
    def NT(self):       # tiles per core
        return self.NBLK * self.TPB


FULL = Cfg()


def small_cfg():
    c = Cfg()
    c.N = 2048
    c.E = 8192
    c.G = 16
    c.NPC = 256
    c.NBLK = 2
    c.BS = 1536        # 12 tiles  (mean edges/block = 8192/16 = 512...)
    c.TPB = 12
    c.CH = 6
    c.NCH = 2
    c.GSPAN = 16
    c.GPAD = 384
    return c


# ---------------------------------------------------------------- program

def build_program(cfg):
    import concourse.bass as bass
    import concourse.bacc as bacc
    import concourse.mybir as mybir
    import concourse.tile as tile
    from contextlib import ExitStack

    f32, bft, i32 = mybir.dt.float32, mybir.dt.bfloat16, mybir.dt.int32
    EQ = mybir.AluOpType.is_equal
    RELU = mybir.ActivationFunctionType.Relu
    COPY = mybir.ActivationFunctionType.Copy
    DM, CH, BS, TPB, NCH, NBLK = cfg.DM, cfg.CH, cfg.BS, cfg.TPB, cfg.NCH, cfg.NBLK
    GSPAN, GPAD = cfg.GSPAN, cfg.GPAD

    nc = bacc.Bacc("TRN2", target_bir_lowering=False, debug=True)

    sA = nc.declare_dram_parameter("sA", [128, cfg.ES], bft, isOutput=False)
    sB = nc.declare_dram_parameter("sB", [64, cfg.ES], bft, isOutput=False)
    dstrelF = nc.declare_dram_parameter("dstrelF", [128, cfg.NT], f32, isOutput=False)
    batchrelF = nc.declare_dram_parameter("batchrelF", [128, NBLK], f32, isOutput=False)
    iotaG = nc.declare_dram_parameter("iotaG", [128, GSPAN], f32, isOutput=False)
    iotaOH = nc.declare_dram_parameter("iotaOH", [128, CH * 128], f32, isOutput=False)
    ident = nc.declare_dram_parameter("ident", [128, 128], f32, isOutput=False)
    Wsd = nc.declare_dram_parameter("Wsd", [128, DM], bft, isOutput=False)
    Wea = nc.declare_dram_parameter("Wea", [128, DM], bft, isOutput=False)
    W1a = nc.declare_dram_parameter("W1a", [64, 20], bft, isOutput=False)
    W2a = nc.declare_dram_parameter("W2a", [64, 10], f32, isOutput=False)
    W3a = nc.declare_dram_parameter("W3a", [64, 1], f32, isOutput=False)
    gmapA = nc.declare_dram_parameter("gmapA", [128, 1], i32, isOutput=False)
    gmapB = nc.declare_dram_parameter("gmapB", [128, 1], i32, isOutput=False)
    out = nc.declare_dram_parameter("out", [1, GPAD], f32, isOutput=True)

    allin = nc.dram_tensor("allin", [GPAD, 20], f32)
    allout = nc.dram_tensor("allout", [GPAD, 20], f32)

    with tile.TileContext(nc) as tc, ExitStack() as xs:
        cp = xs.enter_context(tc.tile_pool(name="const", bufs=1))
        sAp = xs.enter_context(tc.tile_pool(name="sAp", bufs=2))
        sBp = xs.enter_context(tc.tile_pool(name="sBp", bufs=1))
        ohp = xs.enter_context(tc.tile_pool(name="ohp", bufs=2))
        msgp = xs.enter_context(tc.tile_pool(name="msgp", bufs=2))
        smallp = xs.enter_context(tc.tile_pool(name="smallp", bufs=2))
        ps_msg = xs.enter_context(tc.tile_pool(name="ps_msg", bufs=2, space="PSUM"))
        ps_x = xs.enter_context(tc.tile_pool(name="ps_x", bufs=2, space="PSUM"))
        ps_g = xs.enter_context(tc.tile_pool(name="ps_g", bufs=1, space="PSUM"))

        # ---- constants
        dstrel_t = cp.tile([128, cfg.NT], f32)
        nc.sync.dma_start(out=dstrel_t[:], in_=dstrelF[:])
        batchrel_t = cp.tile([128, NBLK], f32)
        nc.sync.dma_start(out=batchrel_t[:], in_=batchrelF[:])
        iotaG_t = cp.tile([128, GSPAN], f32)
        nc.sync.dma_start(out=iotaG_t[:], in_=iotaG[:])
        iotaOH_t = cp.tile([128, CH * 128], f32)
        nc.sync.dma_start(out=iotaOH_t[:], in_=iotaOH[:])
        ident_t = cp.tile([128, 128], f32)
        nc.sync.dma_start(out=ident_t[:], in_=ident[:])
        Wsd_t = cp.tile([128, DM], bft)
        nc.sync.dma_start(out=Wsd_t[:], in_=Wsd[:])
        Wea_t = cp.tile([128, DM], bft)
        nc.sync.dma_start(out=Wea_t[:], in_=Wea[:])
        W1a_t = cp.tile([64, 20], bft)
        nc.sync.dma_start(out=W1a_t[:], in_=W1a[:])
        W2a_t = cp.tile([64, 10], f32)
        nc.sync.dma_start(out=W2a_t[:], in_=W2a[:])
        W3a_t = cp.tile([64, 1], f32)
        nc.sync.dma_start(out=W3a_t[:], in_=W3a[:])
        gmapA_t = cp.tile([128, 1], i32)
        nc.sync.dma_start(out=gmapA_t[:], in_=gmapA[:])
        gmapB_t = cp.tile([128, 1], i32)
        nc.sync.dma_start(out=gmapB_t[:], in_=gmapB[:])

        # ---- zero the AllReduce input buffer
        zz = cp.tile([128, 20], f32)
        nc.vector.memset(zz[:], 0.0)
        for r in range(GPAD // 128):
            nc.sync.dma_start(out=allin[r * 128:(r + 1) * 128, :], in_=zz[:])

        # ---- persistent stream-B tiles (rows 64..127 fixed: ones + zeros)
        sB_tiles = []
        for i in range(2):
            t = sBp.tile([128, BS], bft, tag=f"sB{i}")
            nc.vector.memset(t[64:128, :], 0.0)
            nc.vector.memset(t[64:65, :], 1.0)
            sB_tiles.append(t)

        # ---- pooled-graph accumulators (live across the whole main loop)
        GBW = max(GSPAN - 128, 0)
        gA_ps = ps_g.tile([min(GSPAN, 128), 20], f32, tag="gA")
        if GBW:
            gB_ps = ps_g.tile([GBW, 20], f32, tag="gB")
        else:
            gB_ps = None

        # ---- main loop over 128-node blocks
        for b in range(NBLK):
            sA_t = sAp.tile([128, BS], bft, tag="sA")
            nc.sync.dma_start(out=sA_t[:], in_=sA[:, b * BS:(b + 1) * BS])
            sB_t = sB_tiles[b % 2]
            nc.sync.dma_start(out=sB_t[:64, :], in_=sB[:, b * BS:(b + 1) * BS])

            xblk_ps = ps_x.tile([128, DM], f32, tag="xblk")
            for ch in range(NCH):
                t0 = b * TPB + ch * CH
                msg_ps = ps_msg.tile([128, CH * DM], f32, tag="msgps")
                oh_t = ohp.tile([128, CH * 128], bft, tag="oh")
                dr = dstrel_t[:, t0:t0 + CH].unsqueeze(2).to_broadcast([128, CH, 128])
                nc.vector.tensor_tensor(
                    out=oh_t[:].rearrange("p (a b) -> p a b", b=128),
                    in0=iotaOH_t[:].rearrange("p (a b) -> p a b", b=128),
                    in1=dr,
                    op=EQ,
                )
                for t in range(CH):
                    e0 = (ch * CH + t) * 128
                    nc.tensor.matmul(
                        msg_ps[:, t * DM:(t + 1) * DM],
                        lhsT=sA_t[:, e0:e0 + 128],
                        rhs=Wsd_t[:],
                        start=True, stop=False,
                    )
                    nc.tensor.matmul(
                        msg_ps[:, t * DM:(t + 1) * DM],
                        lhsT=sB_t[:, e0:e0 + 128],
                        rhs=Wea_t[:],
                        start=False, stop=True,
                    )
                msg_t = msgp.tile([128, CH * DM], bft, tag="msg")
                nc.scalar.activation(msg_t[:], msg_ps[:], RELU)
                for t in range(CH):
                    gt = ch * CH + t
                    nc.tensor.matmul(
                        xblk_ps[:],
                        lhsT=oh_t[:, t * 128:(t + 1) * 128],
                        rhs=msg_t[:, t * DM:(t + 1) * DM],
                        start=(gt == 0), stop=(gt == TPB - 1),
                    )

            # block tail: x -> xT -> h = relu(x@W1+b1) -> graph pooling
            xs_t = smallp.tile([128, DM], f32, tag="xs")
            nc.scalar.activation(xs_t[:], xblk_ps[:], COPY)
            xT_ps = ps_msg.tile([DM, 128], f32, tag="xtps")
            nc.tensor.transpose(out=xT_ps[:], in_=xs_t[:], identity=ident_t[:])
            xTa_t = smallp.tile([64, 128], bft, tag="xta")
            nc.vector.memset(xTa_t[:, :], 0.0)
            nc.vector.memset(xTa_t[32:33, :], 1.0)
            nc.vector.tensor_copy(out=xTa_t[:DM, :], in_=xT_ps[:])
            h_ps = ps_msg.tile([128, 20], f32, tag="msgps")
            nc.tensor.matmul(h_ps[:], lhsT=xTa_t[:], rhs=W1a_t[:],
                             start=True, stop=True)
            h_t = smallp.tile([128, 20], bft, tag="h")
            nc.scalar.activation(h_t[:], h_ps[:], RELU)
            ohg_t = smallp.tile([128, GSPAN], bft, tag="ohg")
            nc.vector.tensor_scalar(
                out=ohg_t[:], in0=iotaG_t[:], scalar1=batchrel_t[:, b:b + 1],
                scalar2=None, op0=EQ,
            )
            nc.tensor.matmul(gA_ps[:], lhsT=ohg_t[:, :min(GSPAN, 128)], rhs=h_t[:],
                             start=(b == 0), stop=(b == NBLK - 1))
            if gB_ps is not None:
                nc.tensor.matmul(gB_ps[:], lhsT=ohg_t[:, 128:GSPAN], rhs=h_t[:],
                                 start=(b == 0), stop=(b == NBLK - 1))

        # ---- scatter per-core pooled partials into allin, then AllReduce
        pA_t = cp.tile([128, 20], f32)
        nc.vector.memset(pA_t[:, :], 0.0)
        nc.vector.tensor_copy(out=pA_t[:min(GSPAN, 128), :], in_=gA_ps[:])
        nc.gpsimd.indirect_dma_start(
            out=allin[:], out_offset=bass.IndirectOffsetOnAxis(ap=gmapA_t[:, :1], axis=0),
            in_=pA_t[:], in_offset=None,
        )
        if gB_ps is not None:
            pB_t = cp.tile([128, 20], f32)
            nc.vector.memset(pB_t[:, :], 0.0)
            nc.vector.tensor_copy(out=pB_t[:GBW, :], in_=gB_ps[:])
            nc.gpsimd.indirect_dma_start(
                out=allin[:], out_offset=bass.IndirectOffsetOnAxis(ap=gmapB_t[:, :1], axis=0),
                in_=pB_t[:], in_offset=None,
            )
        nc.gpsimd.collective_compute(
            "AllReduce", mybir.AluOpType.add,
            replica_groups=[list(range(cfg.NCORES))],
            ins=[allin[:]], outs=[allout[:]],
        )

        # ---- graph head (redundant on every core)
        pta = cp.tile([64, GPAD], f32)
        nc.vector.memset(pta[:, :], 0.0)
        nc.vector.memset(pta[32:33, :], 1.0)
        for r in range(GPAD // 128):
            tmp = smallp.tile([128, 20], f32, tag="htmp")
            nc.sync.dma_start(out=tmp[:], in_=allout[r * 128:(r + 1) * 128, :])
            tr_ps = ps_msg.tile([20, 128], f32, tag="xtps")
            nc.tensor.transpose(out=tr_ps[:], in_=tmp[:], identity=ident_t[:])
            nc.vector.tensor_copy(out=pta[:20, r * 128:(r + 1) * 128], in_=tr_ps[:])
        h2a = cp.tile([64, GPAD], f32)
        nc.vector.memset(h2a[:, :], 0.0)
        nc.vector.memset(h2a[32:33, :], 1.0)
        outsb = cp.tile([1, GPAD], f32)
        nchunks = (GPAD + 511) // 512
        for c in range(nchunks):
            lo = c * 512
            hi = min(GPAD, lo + 512)
            h2_ps = ps_msg.tile([10, hi - lo], f32, tag="msgps")
            nc.tensor.matmul(h2_ps[:], lhsT=W2a_t[:], rhs=pta[:, lo:hi],
                             start=True, stop=True)
            nc.scalar.activation(h2a[:10, lo:hi], h2_ps[:], RELU)
            o_ps = ps_msg.tile([1, hi - lo], f32, tag="xtps")
            nc.tensor.matmul(o_ps[:], lhsT=W3a_t[:], rhs=h2a[:, lo:hi],
                             start=True, stop=True)
            nc.vector.tensor_copy(out=outsb[:, lo:hi], in_=o_ps[:])
        nc.sync.dma_start(out=out[:], in_=outsb[:])

    nc.finalize()
    return nc


# ---------------------------------------------------------------- host prep

def host_prep(cfg, edge_index, node_attr, edge_attr, batch,
              W_msg, b_msg, W1, b1, W2, b2, W3, b3):
    N, E, D, G, DM = cfg.N, cfg.E, cfg.D, cfg.G, cfg.DM
    NPC, NBLK, BS, GSPAN, GPAD = cfg.NPC, cfg.NBLK, cfg.BS, cfg.GSPAN, cfg.GPAD
    ES = cfg.ES

    src = np.asarray(edge_index[0], dtype=np.int64).astype(np.int32)
    dst = np.asarray(edge_index[1], dtype=np.int64).astype(np.int32)
    node_attr = np.asarray(node_attr, np.float32)
    edge_attr = np.asarray(edge_attr, np.float32)
    batch = np.asarray(batch, dtype=np.int64).astype(np.int32)

    order = np.argsort(dst, kind="stable")
    src_s, dst_s = src[order], dst[order]
    blk = dst_s // 128                       # global block id, sorted
    nblk_tot = cfg.NCORES * NBLK
    cnt = np.bincount(blk, minlength=nblk_tot)
    assert cnt.max() <= BS, f"block overflow: {cnt.max()} > {BS}"
    starts = np.zeros(nblk_tot, np.int64)
    starts[1:] = np.cumsum(cnt)[:-1]
    rank = np.arange(E) - starts[blk]
    slot = blk.astype(np.int64) * BS + rank   # global slot id in [0, 8*ES)

    na_bf = node_attr.astype(bf16)
    ea_bf = edge_attr.astype(bf16)

    NS_ALL = cfg.NCORES * ES
    A = np.zeros([NS_ALL, 128], bf16)
    A[slot, :64] = na_bf[src_s]
    A[slot, 64:] = na_bf[dst_s]
    B = np.zeros([NS_ALL, 64], bf16)
    B[slot] = ea_bf[order]
    drel = np.full(NS_ALL, -1.0, np.float32)
    drel[slot] = (dst_s % 128).astype(np.float32)

    # per-node graph ids
    g0 = np.zeros(cfg.NCORES, np.int32)
    batchrel = np.zeros([cfg.NCORES, NBLK * 128], np.float32)
    for c in range(cfg.NCORES):
        lo = c * NPC
        hi = min((c + 1) * NPC, N)
        g0[c] = batch[lo]
        rel = np.full(NPC, GSPAN - 1, np.float32)
        rel[:hi - lo] = (batch[lo:hi] - g0[c]).astype(np.float32)
        assert rel.max() <= GSPAN - 1
        batchrel[c] = rel

    iotaG = np.broadcast_to(np.arange(GSPAN, dtype=np.float32), (128, GSPAN)).copy()
    iotaOH = np.broadcast_to(
        np.tile(np.arange(128, dtype=np.float32), cfg.CH), (128, cfg.CH * 128)
    ).copy()
    ident = np.eye(128, dtype=np.float32)

    W_msg = np.asarray(W_msg, np.float32)
    Wsd = W_msg[:128].astype(bf16)
    Wea = np.zeros([128, DM], np.float32)
    Wea[:64] = W_msg[128:192]
    Wea[64] = np.asarray(b_msg, np.float32)
    Wea = Wea.astype(bf16)
    W1a = np.zeros([64, 20], np.float32)
    W1a[:DM] = np.asarray(W1, np.float32)
    W1a[32] = np.asarray(b1, np.float32)
    W1a = W1a.astype(bf16)
    W2a = np.zeros([64, 10], np.float32)
    W2a[:20] = np.asarray(W2, np.float32)
    W2a[32] = np.asarray(b2, np.float32)
    W3a = np.zeros([64, 1], np.float32)
    W3a[:10] = np.asarray(W3, np.float32)
    W3a[32] = np.asarray(b3, np.float32)

    in_maps = []
    for c in range(cfg.NCORES):
        sl = slice(c * ES, (c + 1) * ES)
        gmA = np.zeros([128, 1], np.int32)
        gmB = np.zeros([128, 1], np.int32)
        for i in range(128):
            gi = g0[c] + i
            gmA[i, 0] = gi if (gi < G and i < GSPAN) else GPAD - 256 + i
        for i in range(128):
            j = 128 + i
            gj = g0[c] + j
            gmB[i, 0] = gj if (gj < G and j < GSPAN) else GPAD - 128 + i
        assert gmA.max() < GPAD and gmB.max() < GPAD
        in_maps.append({
            "sA": np.ascontiguousarray(A[sl].T),
            "sB": np.ascontiguousarray(B[sl].T),
            "dstrelF": np.ascontiguousarray(drel[sl].reshape(cfg.NT, 128).T),
            "batchrelF": np.ascontiguousarray(batchrel[c].reshape(NBLK, 128).T),
            "iotaG": iotaG,
            "iotaOH": iotaOH,
            "ident": ident,
            "Wsd": Wsd, "Wea": Wea, "W1a": W1a, "W2a": W2a, "W3a": W3a,
            "gmapA": gmA, "gmapB": gmB,
        })
    return in_maps


# ---------------------------------------------------------------- kernel

_CACHE = {}


def _get_program(cfg):
    key = (cfg.N, cfg.E, cfg.BS)
    if key not in _CACHE:
        _CACHE[key] = build_program(cfg)
    return _CACHE[key]


last_exec_ns = None


def kernel(**inputs):
    import os
    from concourse.bass_utils import run_bass_kernel_spmd

    global last_exec_ns
    cfg = FULL
    nc = _get_program(cfg)
    in_maps = host_prep(cfg, **inputs)
    trace = bool(os.environ.get("GNN_TRACE"))
    res = run_bass_kernel_spmd(nc, in_maps, list(range(cfg.NCORES)), trace=trace)
    last_exec_ns = res.exec_time_ns
    out = np.asarray(res.results[0]["out"]).reshape(-1)[:cfg.G]
    return out.reshape(cfg.G, 1).astype(np.float32)


# revision 9
# speedup vs baseline: 1.3458x; 1.3458x over previous
"""GNN message-passing kernel for 8 trn2 NeuronCores (Bass/Tile).

Model (reference):
    msg  = relu(concat(x[src], x[dst], e_attr) @ W_msg + b_msg)   # [E, 30]
    x1   = segment_sum(msg, dst, N)                                # [N, 30]
    h    = relu(x1 @ W1 + b1)                                      # [N, 20]
    g    = segment_sum(h, batch, G)                                # [G, 20]
    out  = relu(g @ W2 + b2) @ W3 + b3                             # [G, 1]

Two-pass scheme:
  pass 1: each core computes P = node_attr @ W_src and Q = node_attr @
          W_dst for its 1/8 node range (output PQ [NPC, 64] bf16).
  host:   gathers PQ rows per edge endpoint (the "replicated node
          table" gather of the sharding strategy) and assembles one
          fused per-edge stream sM [128, slots]: rows 0-29 P[src],
          30-59 Q[dst], row 60 ones (bias), rows 64-127 e_attr^T.
  pass 2: edges are bucketed by dst (12544 nodes / 98 blocks of 128
          per core, each block statically padded to 2304 slots).  One
          matmul per 128-edge tile produces the messages (lhsT =
          stream tile, rhs = Wc with stacked I30/I30/b/W_e), relu on
          ACT, then a one-hot scatter matmul accumulates per-block
          node sums in PSUM.  Per-node MLP + graph pooling run per
          block; pooled per-graph partials are scattered into a
          [GPAD, 20] buffer, AllReduced across cores, and the tiny
          graph head runs redundantly on every core.
"""
import sys

if "/opt/trn_rl_repo" not in sys.path:
    sys.path.insert(0, "/opt/trn_rl_repo")

import numpy as np
import ml_dtypes

bf16 = ml_dtypes.bfloat16

# ---------------------------------------------------------------- config

class Cfg:
    N = 100000          # nodes
    E = 1600000         # edges
    D = 64              # feature dim
    G = 1000            # graphs
    DM = 30             # message dim
    NCORES = 8
    NPC = 12544         # nodes per core (98 * 128)
    NBLK = 98           # 128-node blocks per core
    BS = 2304           # edge slots per block (18 tiles)
    TPB = 18            # tiles per block
    CH = 9              # tiles per chunk
    NCH = 2             # chunks per block
    GSPAN = 192         # per-core graph window (incl. trash slots)
    GPAD = 1280         # padded global graph rows (1000 real + trash)

    @property
    def ES(self):
        return self.NBLK * self.BS

    @property
    def NT(self):
        return self.NBLK * self.TPB


FULL = Cfg()


def small_cfg():
    c = Cfg()
    c.N = 2048
    c.E = 8192
    c.G = 16
    c.NPC = 256
    c.NBLK = 2
    c.BS = 1536
    c.TPB = 12
    c.CH = 6
    c.NCH = 2
    c.GSPAN = 16
    c.GPAD = 384
    return c


# ---------------------------------------------------------------- pass 1

def build_pq_program(cfg):
    import concourse.bacc as bacc
    import concourse.mybir as mybir
    import concourse.tile as tile
    from contextlib import ExitStack

    f32, bft = mybir.dt.float32, mybir.dt.bfloat16
    COPY = mybir.ActivationFunctionType.Copy
    NPC = cfg.NPC
    NTL = NPC // 128

    nc = bacc.Bacc("TRN2", target_bir_lowering=False, debug=True)
    naT = nc.declare_dram_parameter("naT", [64, NPC], bft, isOutput=False)
    Wpq = nc.declare_dram_parameter("Wpq", [64, 64], bft, isOutput=False)
    PQ = nc.declare_dram_parameter("PQ", [NPC, 64], bft, isOutput=True)

    with tile.TileContext(nc) as tc, ExitStack() as xs:
        cp = xs.enter_context(tc.tile_pool(name="const", bufs=1))
        ps = xs.enter_context(tc.tile_pool(name="ps", bufs=2, space="PSUM"))
        naT_t = cp.tile([64, NPC], bft)
        nc.sync.dma_start(out=naT_t[:], in_=naT[:])
        Wpq_t = cp.tile([64, 64], bft)
        nc.sync.dma_start(out=Wpq_t[:], in_=Wpq[:])
        acc = cp.tile([128, NTL * 64], bft)
        for t in range(NTL):
            pq_ps = ps.tile([128, 64], f32, tag="pq")
            nc.tensor.matmul(pq_ps[:], lhsT=naT_t[:, t * 128:(t + 1) * 128],
                             rhs=Wpq_t[:], start=True, stop=True)
            nc.scalar.activation(acc[:, t * 64:(t + 1) * 64], pq_ps[:], COPY)
        nc.sync.dma_start(
            out=PQ.rearrange("(t p) c -> p t c", p=128),
            in_=acc[:].rearrange("p (t c) -> p t c", c=64),
        )
    nc.finalize()
    return nc


# ---------------------------------------------------------------- pass 2

def build_main_program(cfg):
    import concourse.bass as bass
    import concourse.bacc as bacc
    import concourse.mybir as mybir
    import concourse.tile as tile
    from contextlib import ExitStack

    f32, bft, i32 = mybir.dt.float32, mybir.dt.bfloat16, mybir.dt.int32
    EQ = mybir.AluOpType.is_equal
    RELU = mybir.ActivationFunctionType.Relu
    COPY = mybir.ActivationFunctionType.Copy
    DM, CH, BS, TPB, NCH, NBLK = cfg.DM, cfg.CH, cfg.BS, cfg.TPB, cfg.NCH, cfg.NBLK
    GSPAN, GPAD = cfg.GSPAN, cfg.GPAD

    nc = bacc.Bacc("TRN2", target_bir_lowering=False, debug=True)

    sM = nc.declare_dram_parameter("sM", [NBLK * 128, BS], bft, isOutput=False)
    dstrelF = nc.declare_dram_parameter("dstrelF", [128, cfg.NT], bft, isOutput=False)
    batchrelF = nc.declare_dram_parameter("batchrelF", [128, NBLK], f32, isOutput=False)
    iotaG = nc.declare_dram_parameter("iotaG", [128, GSPAN], bft, isOutput=False)
    iotaOH = nc.declare_dram_parameter("iotaOH", [128, CH * 128], bft, isOutput=False)
    ident = nc.declare_dram_parameter("ident", [128, 128], f32, isOutput=False)
    Wc = nc.declare_dram_parameter("Wc", [128, DM], bft, isOutput=False)
    W1a = nc.declare_dram_parameter("W1a", [64, 20], bft, isOutput=False)
    W2a = nc.declare_dram_parameter("W2a", [64, 10], f32, isOutput=False)
    W3a = nc.declare_dram_parameter("W3a", [64, 1], f32, isOutput=False)
    gmapA = nc.declare_dram_parameter("gmapA", [128, 1], i32, isOutput=False)
    gmapB = nc.declare_dram_parameter("gmapB", [128, 1], i32, isOutput=False)
    out = nc.declare_dram_parameter("out", [1, GPAD], f32, isOutput=True)

    allin = nc.dram_tensor("allin", [GPAD, 20], f32)
    allout = nc.dram_tensor("allout", [GPAD, 20], f32)

    with tile.TileContext(nc) as tc, ExitStack() as xs:
        cp = xs.enter_context(tc.tile_pool(name="const", bufs=1))
        sMp = xs.enter_context(tc.tile_pool(name="sMp", bufs=3))
        ohp = xs.enter_context(tc.tile_pool(name="ohp", bufs=2))
        msgp = xs.enter_context(tc.tile_pool(name="msgp", bufs=2))
        smallp = xs.enter_context(tc.tile_pool(name="smallp", bufs=2))
        ps_msg = xs.enter_context(tc.tile_pool(name="ps_msg", bufs=2, space="PSUM"))
        ps_x = xs.enter_context(tc.tile_pool(name="ps_x", bufs=2, space="PSUM"))
        ps_g = xs.enter_context(tc.tile_pool(name="ps_g", bufs=1, space="PSUM"))

        # ---- constants
        dstrel_t = cp.tile([128, cfg.NT], bft)
        nc.sync.dma_start(out=dstrel_t[:], in_=dstrelF[:])
        batchrel_t = cp.tile([128, NBLK], f32)
        nc.sync.dma_start(out=batchrel_t[:], in_=batchrelF[:])
        iotaG_t = cp.tile([128, GSPAN], bft)
        nc.sync.dma_start(out=iotaG_t[:], in_=iotaG[:])
        iotaOH_t = cp.tile([128, CH * 128], bft)
        nc.sync.dma_start(out=iotaOH_t[:], in_=iotaOH[:])
        ident_t = cp.tile([128, 128], f32)
        nc.sync.dma_start(out=ident_t[:], in_=ident[:])
        Wc_t = cp.tile([128, DM], bft)
        nc.sync.dma_start(out=Wc_t[:], in_=Wc[:])
        W1a_t = cp.tile([64, 20], bft)
        nc.sync.dma_start(out=W1a_t[:], in_=W1a[:])
        W2a_t = cp.tile([64, 10], f32)
        nc.sync.dma_start(out=W2a_t[:], in_=W2a[:])
        W3a_t = cp.tile([64, 1], f32)
        nc.sync.dma_start(out=W3a_t[:], in_=W3a[:])
        gmapA_t = cp.tile([128, 1], i32)
        nc.sync.dma_start(out=gmapA_t[:], in_=gmapA[:])
        gmapB_t = cp.tile([128, 1], i32)
        nc.sync.dma_start(out=gmapB_t[:], in_=gmapB[:])

        # ---- zero the AllReduce input buffer
        zz = cp.tile([128, 20], f32)
        nc.vector.memset(zz[:], 0.0)
        for r in range(GPAD // 128):
            nc.sync.dma_start(out=allin[r * 128:(r + 1) * 128, :], in_=zz[:])

        # ---- pooled-graph accumulators (live across the whole main loop)
        GBW = max(GSPAN - 128, 0)
        gA_ps = ps_g.tile([min(GSPAN, 128), 20], f32, tag="gA")
        if GBW:
            gB_ps = ps_g.tile([GBW, 20], f32, tag="gB")
        else:
            gB_ps = None

        # ---- main loop over 128-node blocks
        for b in range(NBLK):
            sM_t = sMp.tile([128, BS], bft, tag="sM")
            nc.sync.dma_start(out=sM_t[:], in_=sM[b * 128:(b + 1) * 128, :])

            xblk_ps = ps_x.tile([128, DM], f32, tag="xblk")
            for ch in range(NCH):
                t0 = b * TPB + ch * CH
                msg_ps = ps_msg.tile([128, CH * DM], f32, tag="msgps")
                oh_t = ohp.tile([128, CH * 128], bft, tag="oh")
                dr = dstrel_t[:, t0:t0 + CH].unsqueeze(2).to_broadcast([128, CH, 128])
                nc.vector.tensor_tensor(
                    out=oh_t[:].rearrange("p (a b) -> p a b", b=128),
                    in0=iotaOH_t[:].rearrange("p (a b) -> p a b", b=128),
                    in1=dr,
                    op=EQ,
                )
                for t in range(CH):
                    e0 = (ch * CH + t) * 128
                    nc.tensor.matmul(
                        msg_ps[:, t * DM:(t + 1) * DM],
                        lhsT=sM_t[:, e0:e0 + 128],
                        rhs=Wc_t[:],
                        start=True, stop=True,
                    )
                msg_t = msgp.tile([128, CH * DM], bft, tag="msg")
                nc.scalar.activation(msg_t[:], msg_ps[:], RELU)
                for t in range(CH):
                    gt = ch * CH + t
                    nc.tensor.matmul(
                        xblk_ps[:],
                        lhsT=oh_t[:, t * 128:(t + 1) * 128],
                        rhs=msg_t[:, t * DM:(t + 1) * DM],
                        start=(gt == 0), stop=(gt == TPB - 1),
                    )

            # block tail: x -> xT -> h = relu(x@W1+b1) -> graph pooling
            xs_t = smallp.tile([128, DM], f32, tag="xs")
            nc.scalar.activation(xs_t[:], xblk_ps[:], COPY)
            xT_ps = ps_msg.tile([DM, 128], f32, tag="xtps")
            nc.tensor.transpose(out=xT_ps[:], in_=xs_t[:], identity=ident_t[:])
            xTa_t = smallp.tile([64, 128], bft, tag="xta")
            nc.vector.memset(xTa_t[:, :], 0.0)
            nc.vector.memset(xTa_t[32:33, :], 1.0)
            nc.vector.tensor_copy(out=xTa_t[:DM, :], in_=xT_ps[:])
            h_ps = ps_msg.tile([128, 20], f32, tag="msgps")
            nc.tensor.matmul(h_ps[:], lhsT=xTa_t[:], rhs=W1a_t[:],
                             start=True, stop=True)
            h_t = smallp.tile([128, 20], bft, tag="h")
            nc.scalar.activation(h_t[:], h_ps[:], RELU)
            ohg_t = smallp.tile([128, GSPAN], bft, tag="ohg")
            nc.vector.tensor_scalar(
                out=ohg_t[:], in0=iotaG_t[:], scalar1=batchrel_t[:, b:b + 1],
                scalar2=None, op0=EQ,
            )
            nc.tensor.matmul(gA_ps[:], lhsT=ohg_t[:, :min(GSPAN, 128)], rhs=h_t[:],
                             start=(b == 0), stop=(b == NBLK - 1))
            if gB_ps is not None:
                nc.tensor.matmul(gB_ps[:], lhsT=ohg_t[:, 128:GSPAN], rhs=h_t[:],
                                 start=(b == 0), stop=(b == NBLK - 1))

        # ---- scatter per-core pooled partials into allin, then AllReduce
        pA_t = cp.tile([128, 20], f32)
        nc.vector.memset(pA_t[:, :], 0.0)
        nc.vector.tensor_copy(out=pA_t[:min(GSPAN, 128), :], in_=gA_ps[:])
        nc.gpsimd.indirect_dma_start(
            out=allin[:], out_offset=bass.IndirectOffsetOnAxis(ap=gmapA_t[:, :1], axis=0),
            in_=pA_t[:], in_offset=None,
        )
        if gB_ps is not None:
            pB_t = cp.tile([128, 20], f32)
            nc.vector.memset(pB_t[:, :], 0.0)
            nc.vector.tensor_copy(out=pB_t[:GBW, :], in_=gB_ps[:])
            nc.gpsimd.indirect_dma_start(
                out=allin[:], out_offset=bass.IndirectOffsetOnAxis(ap=gmapB_t[:, :1], axis=0),
                in_=pB_t[:], in_offset=None,
            )
        nc.gpsimd.collective_compute(
            "AllReduce", mybir.AluOpType.add,
            replica_groups=[list(range(cfg.NCORES))],
            ins=[allin[:]], outs=[allout[:]],
        )

        # ---- graph head (redundant on every core)
        pta = cp.tile([64, GPAD], f32)
        nc.vector.memset(pta[:, :], 0.0)
        nc.vector.memset(pta[32:33, :], 1.0)
        for r in range(GPAD // 128):
            tmp = smallp.tile([128, 20], f32, tag="htmp")
            nc.sync.dma_start(out=tmp[:], in_=allout[r * 128:(r + 1) * 128, :])
            tr_ps = ps_msg.tile([20, 128], f32, tag="xtps")
            nc.tensor.transpose(out=tr_ps[:], in_=tmp[:], identity=ident_t[:])
            nc.vector.tensor_copy(out=pta[:20, r * 128:(r + 1) * 128], in_=tr_ps[:])
        h2a = cp.tile([64, GPAD], f32)
        nc.vector.memset(h2a[:, :], 0.0)
        nc.vector.memset(h2a[32:33, :], 1.0)
        outsb = cp.tile([1, GPAD], f32)
        nchunks = (GPAD + 511) // 512
        for c in range(nchunks):
            lo = c * 512
            hi = min(GPAD, lo + 512)
            h2_ps = ps_msg.tile([10, hi - lo], f32, tag="msgps")
            nc.tensor.matmul(h2_ps[:], lhsT=W2a_t[:], rhs=pta[:, lo:hi],
                             start=True, stop=True)
            nc.scalar.activation(h2a[:10, lo:hi], h2_ps[:], RELU)
            o_ps = ps_msg.tile([1, hi - lo], f32, tag="xtps")
            nc.tensor.matmul(o_ps[:], lhsT=W3a_t[:], rhs=h2a[:, lo:hi],
                             start=True, stop=True)
            nc.vector.tensor_copy(out=outsb[:, lo:hi], in_=o_ps[:])
        nc.sync.dma_start(out=out[:], in_=outsb[:])

    nc.finalize()
    return nc


# ---------------------------------------------------------------- host prep

def host_plan(cfg, edge_index, batch):
    """Slot assignment + graph-window metadata (independent of features)."""
    N, E, G = cfg.N, cfg.E, cfg.G
    NPC, NBLK, BS, GSPAN = cfg.NPC, cfg.NBLK, cfg.BS, cfg.GSPAN

    src = np.asarray(edge_index[0]).astype(np.int64)
    dst = np.asarray(edge_index[1]).astype(np.int64)
    batch = np.asarray(batch).astype(np.int32)

    order = np.argsort(dst, kind="stable")
    src_s, dst_s = src[order], dst[order]
    blk = dst_s // 128
    nblk_tot = cfg.NCORES * NBLK
    cnt = np.bincount(blk, minlength=nblk_tot)
    assert cnt.max() <= BS, f"block overflow: {cnt.max()} > {BS}"
    starts = np.zeros(nblk_tot, np.int64)
    starts[1:] = np.cumsum(cnt)[:-1]
    rank = np.arange(E) - starts[blk]
    slot = blk * BS + rank

    NS_ALL = cfg.NCORES * cfg.ES
    drel = np.full(NS_ALL, -1.0, np.float32)
    drel[slot] = (dst_s % 128).astype(np.float32)

    g0 = np.zeros(cfg.NCORES, np.int32)
    batchrel = np.zeros([cfg.NCORES, NPC], np.float32)
    for c in range(cfg.NCORES):
        lo = c * NPC
        hi = min((c + 1) * NPC, N)
        g0[c] = batch[lo]
        rel = np.full(NPC, GSPAN - 1, np.float32)
        rel[:hi - lo] = (batch[lo:hi] - g0[c]).astype(np.float32)
        assert rel.max() <= GSPAN - 1
        batchrel[c] = rel

    return dict(order=order, src_s=src_s, dst_s=dst_s, slot=slot, drel=drel,
                g0=g0, batchrel=batchrel)


def host_prep_pq(cfg, node_attr, W_msg):
    naT = np.zeros([64, cfg.NCORES * cfg.NPC], bf16)
    naT[:, :cfg.N] = np.asarray(node_attr, np.float32).astype(bf16).T
    W_msg = np.asarray(W_msg, np.float32)
    Wpq = np.zeros([64, 64], np.float32)
    Wpq[:, 0:cfg.DM] = W_msg[0:64]
    Wpq[:, cfg.DM:2 * cfg.DM] = W_msg[64:128]
    Wpq = Wpq.astype(bf16)
    in_maps = []
    for c in range(cfg.NCORES):
        in_maps.append({
            "naT": np.ascontiguousarray(naT[:, c * cfg.NPC:(c + 1) * cfg.NPC]),
            "Wpq": Wpq,
        })
    return in_maps


def host_prep_main(cfg, plan, PQ_full, edge_attr, W_msg, b_msg,
                   W1, b1, W2, b2, W3, b3):
    G, DM = cfg.G, cfg.DM
    NBLK, BS, GSPAN, GPAD = cfg.NBLK, cfg.BS, cfg.GSPAN, cfg.GPAD
    ES = cfg.ES
    NS_ALL = cfg.NCORES * ES

    ea_bf = np.asarray(edge_attr, np.float32).astype(bf16)
    slot, order = plan["slot"], plan["order"]
    src_s, dst_s = plan["src_s"], plan["dst_s"]

    M = np.zeros([NS_ALL, 128], bf16)
    M[slot, 0:DM] = PQ_full[src_s, 0:DM]
    M[slot, DM:2 * DM] = PQ_full[dst_s, DM:2 * DM]
    M[:, 60] = bf16(1.0)
    M[slot, 64:128] = ea_bf[order]

    iotaG = np.broadcast_to(
        np.arange(GSPAN, dtype=np.float32), (128, GSPAN)).astype(bf16)
    iotaOH = np.broadcast_to(
        np.tile(np.arange(128, dtype=np.float32), cfg.CH), (128, cfg.CH * 128)
    ).astype(bf16)
    ident = np.eye(128, dtype=np.float32)

    W_msg = np.asarray(W_msg, np.float32)
    Wcm = np.zeros([128, DM], np.float32)
    Wcm[0:DM] = np.eye(DM)
    Wcm[DM:2 * DM] = np.eye(DM)
    Wcm[60] = np.asarray(b_msg, np.float32)
    Wcm[64:128] = W_msg[128:192]
    Wcm = Wcm.astype(bf16)
    W1a = np.zeros([64, 20], np.float32)
    W1a[:DM] = np.asarray(W1, np.float32)
    W1a[32] = np.asarray(b1, np.float32)
    W1a = W1a.astype(bf16)
    W2a = np.zeros([64, 10], np.float32)
    W2a[:20] = np.asarray(W2, np.float32)
    W2a[32] = np.asarray(b2, np.float32)
    W3a = np.zeros([64, 1], np.float32)
    W3a[:10] = np.asarray(W3, np.float32)
    W3a[32] = np.asarray(b3, np.float32)

    g0 = plan["g0"]
    in_maps = []
    for c in range(cfg.NCORES):
        Mc = M[c * ES:(c + 1) * ES].reshape(NBLK, BS, 128)
        Mc = np.ascontiguousarray(Mc.swapaxes(1, 2)).reshape(NBLK * 128, BS)
        gmA = np.zeros([128, 1], np.int32)
        gmB = np.zeros([128, 1], np.int32)
        for i in range(128):
            gi = g0[c] + i
            gmA[i, 0] = gi if (gi < G and i < GSPAN) else GPAD - 256 + i
        for i in range(128):
            j = 128 + i
            gj = g0[c] + j
            gmB[i, 0] = gj if (gj < G and j < GSPAN) else GPAD - 128 + i
        assert gmA.max() < GPAD and gmB.max() < GPAD
        sl = slice(c * ES, (c + 1) * ES)
        in_maps.append({
            "sM": Mc,
            "dstrelF": np.ascontiguousarray(
                plan["drel"][sl].reshape(cfg.NT, 128).T).astype(bf16),
            "batchrelF": np.ascontiguousarray(
                plan["batchrel"][c].reshape(NBLK, 128).T),
            "iotaG": iotaG,
            "iotaOH": iotaOH,
            "ident": ident,
            "Wc": Wcm, "W1a": W1a, "W2a": W2a, "W3a": W3a,
            "gmapA": gmA, "gmapB": gmB,
        })
    return in_maps


# ---------------------------------------------------------------- kernel

_CACHE = {}


def _get_programs(cfg):
    key = (cfg.N, cfg.E, cfg.BS)
    if key not in _CACHE:
        _CACHE[key] = (build_pq_program(cfg), build_main_program(cfg))
    return _CACHE[key]


last_exec_ns = None
last_exec_ns_pq = None


def _run(cfg, inputs):
    import os
    from concourse.bass_utils import run_bass_kernel_spmd

    global last_exec_ns, last_exec_ns_pq
    nc_pq, nc_main = _get_programs(cfg)
    trace = bool(os.environ.get("GNN_TRACE"))

    plan = host_plan(cfg, inputs["edge_index"], inputs["batch"])
    pq_maps = host_prep_pq(cfg, inputs["node_attr"], inputs["W_msg"])
    res1 = run_bass_kernel_spmd(nc_pq, pq_maps, list(range(cfg.NCORES)),
                                trace=trace)
    PQ_full = np.concatenate(
        [np.asarray(res1.results[c]["PQ"]) for c in range(cfg.NCORES)], axis=0
    )
    last_exec_ns_pq = res1.exec_time_ns

    in_maps = host_prep_main(
        cfg, plan, PQ_full, inputs["edge_attr"], inputs["W_msg"],
        inputs["b_msg"], inputs["W1"], inputs["b1"], inputs["W2"],
        inputs["b2"], inputs["W3"], inputs["b3"],
    )
    res = run_bass_kernel_spmd(nc_main, in_maps, list(range(cfg.NCORES)),
                               trace=trace)
    last_exec_ns = res.exec_time_ns
    out = np.asarray(res.results[0]["out"]).reshape(-1)[:cfg.G]
    return out.reshape(cfg.G, 1).astype(np.float32)


def kernel(**inputs):
    return _run(FULL, inputs)
